# revision 58
# baseline (speedup 1.0000x reference)
"""GAT message-passing kernel for 8 Trainium2 NeuronCores (Bass/Tile).

Strategy (edge-parallel, dst-routed; v7):
  * Host: sort edges by destination, partition dst nodes into 8 cores x 50
    blocks of 128.  Each core owns the full segment-softmax + scatter-add
    for its dst range (no cross-core collectives).
  * Phase A (device, replicated): project x -> [k||v] for all nodes into a
    bf16 HBM table (v columns stored (d,h)-interleaved; xT columns permuted
    so each partition stores 8 consecutive table rows = 4KB-contiguous
    store descriptors); project x -> q for the core's local nodes straight
    into SBUF.  hi-table groups are written first so the per-superblock hi
    gather (issued first) can start ~2/3 into phase A.
  * Phase B per superblock: dma_gather kv rows by edge src in bf16 (lo/hi
    tables because gather indices are int16).  Per 128-edge chunk:
      - P^T comes precomputed from the host via HWDGE DMA; P (for the agg
        matmul) is derived on-chip by PE transpose + scalar copy.
      - qsel = P^T.T @ q_sb on PE; copied to bf16 SBUF so the qk multiply
        runs in the DVE 2x perf mode.
      - score reduce outputs bf16 (all-2B operands -> DVE 2x reduce;
        internal accumulation stays f32).
      - msg = v * exp with v (d,h)-interleaved so the exp broadcast has
        stride-1 inner axis (2x DVE mode); Wout rows permuted to match.
      - one PE matmul per chunk accumulates agg[j,:] and denom[j,h] in PSUM.
  * Block epilogue: agg/denom, PE transpose, @Wout + bias, relu, +x, store.

The single Bass program is shared by all 8 cores (SPMD); everything
data-dependent arrives as per-core input tensors with uniform shapes.
"""

import numpy as np
import ml_dtypes

# ----- problem constants (hardcoded per contest rules) -----
N = 50000
E = 800000
D = 128          # IN_DIM == OUT_DIM == HEADS*HEAD_DIM
H = 4
HD = 32
BLK = 128
LO_LIMIT = 32768     # int16 gather-index limit (lo table = rows [0, 32768))
HI_BASE = 18432      # hi table = rows [18432, 51200), width 32768

bf16 = ml_dtypes.bfloat16

# v columns / Wout rows are stored (d, h)-interleaved: new col d*H+h holds
# old col h*HD+d.  Gives the msg-multiply a stride-1 inner axis on the
# broadcast exp operand.
VPERM = np.empty(D, np.int64)
for _h in range(H):
    for _d in range(HD):
        VPERM[_d * H + _h] = _h * HD + _d


def _ceil_div(a, b):
    return (a + b - 1) // b


def _wrap16(stream_i16):
    """Pack a flat descriptor-index stream into the [128, n/16] SBUF layout
    dma_gather expects (idx i at [i%16, i//16], replicated to all 8
    16-partition groups)."""
    n = stream_i16.shape[0]
    assert n % 16 == 0
    a = stream_i16.reshape(n // 16, 16).T  # [16, n/16]
    return np.tile(a, (8, 1)).astype(np.int16)  # [128, n/16]


def _prep(x, edge_index, Wt, Ws, Wc, Wout, bout, ncores, nbc, sbb):
    """Host-side preprocessing: sort/route edges, build all per-core arrays."""
    npb = ncores * nbc              # total padded blocks
    npad = npb * BLK                # padded node count
    nnc = nbc * BLK                 # nodes per core
    nsb = nbc // sbb                # superblocks per core
    assert nbc % sbb == 0

    x = np.asarray(x, np.float32)
    n = x.shape[0]
    src = np.asarray(edge_index[0]).astype(np.int64)
    dst = np.asarray(edge_index[1]).astype(np.int64)

    perm = np.arange(npad)
    inv_perm = np.arange(npad)
    dstp = perm[dst]
    order = np.argsort(dstp, kind="stable")
    src_s = src[order].astype(np.int32)
    dst_s = dstp[order].astype(np.int32)

    bounds = np.searchsorted(dst_s, np.arange(0, npad + 1, BLK)).astype(np.int64)
    # overlap-balanced lo/hi split: srcs in [HI_BASE, LO_LIMIT) can go either
    # way.  Pick a global lo-chunk budget minimizing cpb_lo + cpb_hi.
    blocks = []
    for b in range(npb):
        s, e = bounds[b], bounds[b + 1]
        bs, bd = src_s[s:e], dst_s[s:e] % BLK
        blocks.append((bs, bd, int((bs < HI_BASE).sum()),
                       int((bs < LO_LIMIT).sum())))
    best = None
    for tl in range(1, 20):
        cap = tl * BLK
        cl = ch_ = 1
        for bs, bd, must_lo, can_lo in blocks:
            nlo = min(max(cap, must_lo), can_lo)
            cl = max(cl, _ceil_div(nlo, BLK))
            ch_ = max(ch_, _ceil_div(len(bs) - nlo, BLK))
        if cl <= tl and (best is None or cl + ch_ < best[0] + best[1]):
            best = (cl, ch_, cap)
    cpb_lo, cpb_hi, cap = best
    lo_list, hi_list = [], []   # per block: (table_idx, dst_local)
    for bs, bd, must_lo, can_lo in blocks:
        nlo = min(max(cap, must_lo), can_lo)
        is_flex = (bs >= HI_BASE) & (bs < LO_LIMIT)
        lo_mask = bs < HI_BASE
        take_flex = nlo - must_lo
        lo_mask[np.where(is_flex)[0][:take_flex]] = True
        lo_list.append((bs[lo_mask], bd[lo_mask]))
        hi_list.append((bs[~lo_mask] - HI_BASE, bd[~lo_mask]))
    ch = sbb * (cpb_lo + cpb_hi)    # chunks per superblock
    wlo = sbb * cpb_lo * BLK        # lo edge slots per superblock
    whi = sbb * cpb_hi * BLK

    # padded x / weights
    xpad = np.zeros((npad, D), np.float32)
    xpad[:n] = x
    # xT columns permuted so that phase-A group g, matmul b covers nodes
    # {g*1024 + p*8 + b : p in 0..128} -> the kv store writes 8 consecutive
    # table rows (4KB contiguous) per partition instead of 8x 512B strided.
    GA = 16
    colperm = np.arange(npad).reshape(npad // (GA * BLK), BLK, GA)
    colperm = colperm.transpose(0, 2, 1).reshape(-1)  # [g, b, p] -> node
    xT_bf = np.ascontiguousarray(xpad.T[:, colperm]).astype(bf16)
    xpad_perm = np.zeros((npad, D), np.float32)
    ok = inv_perm < n
    xpad_perm[ok] = xpad[inv_perm[ok]]
    WcP = np.asarray(Wc, np.float32)[:, VPERM]             # (d,h)-interleaved v
    Wskvc = np.ascontiguousarray(
        np.concatenate([np.asarray(Ws, np.float32), WcP], axis=1)).astype(bf16)
    WoutP = np.ascontiguousarray(np.asarray(Wout, np.float32)[VPERM, :])
    ident = np.eye(BLK, dtype=np.float32).astype(bf16)

    in_maps = []
    for c in range(ncores):
        first_node = c * nnc
        kvlo = np.zeros((nsb, wlo), np.int16)
        kvhi = np.zeros((nsb, whi), np.int16)
        dstl = np.full((nsb, ch * BLK), -1.0, np.float32)
        for s in range(nsb):
            for bb in range(sbb):
                gb = c * nbc + s * sbb + bb            # global block
                (ls, ld), (hs, hd_) = lo_list[gb], hi_list[gb]
                # lo segment: chunks [bb*cpb_lo, (bb+1)*cpb_lo)
                o = bb * cpb_lo * BLK
                kvlo[s, o:o + len(ls)] = ls
                dstl[s, o:o + len(ld)] = ld
                # hi segment: chunks [sbb*cpb_lo + bb*cpb_hi, ...)
                o = bb * cpb_hi * BLK
                kvhi[s, o:o + len(hs)] = hs
                o = (sbb * cpb_lo + bb * cpb_hi) * BLK
                dstl[s, o:o + len(hd_)] = hd_
        kvlo_w = np.concatenate([_wrap16(kvlo[s]) for s in range(nsb)], axis=1)
        kvhi_w = np.concatenate([_wrap16(kvhi[s]) for s in range(nsb)], axis=1)
        # host-precomputed one-hots.
        # P^T: [j, (s*ch + c)*128 + e];  P: [e, (s*ch + c)*128 + j]
        flat = dstl.reshape(-1)
        cols = np.arange(flat.shape[0])
        valid = flat >= 0
        ptT = np.zeros((BLK, nsb * ch * BLK), np.float32)
        ptT[flat[valid].astype(np.int64), cols[valid]] = 1.0

        in_maps.append({
            "xT_full": xT_bf,
            "x_local": np.ascontiguousarray(
                xpad_perm[first_node:first_node + nnc]).astype(bf16),
            "kvlo_idx": np.ascontiguousarray(kvlo_w),
            "kvhi_idx": np.ascontiguousarray(kvhi_w),
            "ptT": np.ascontiguousarray(ptT).astype(bf16),
            "Wskvc": Wskvc,
            "Wt": np.ascontiguousarray(np.asarray(Wt, np.float32)).astype(bf16),
            "Wout": np.ascontiguousarray(WoutP).astype(bf16),
            "ident": ident,
            "delta0": np.ascontiguousarray(
                np.eye(BLK, 1, dtype=np.float32) @ np.ones((1, BLK), np.float32)
            ).T.astype(bf16),
            "bias_row": np.concatenate(
                [np.asarray(bout, np.float32)[None, :],
                 np.zeros((BLK - 1, BLK), np.float32)]).astype(bf16),
        })

    meta = dict(ncores=ncores, nbc=nbc, sbb=sbb, nsb=nsb, npb=npb, npad=npad,
                nnc=nnc, cpb_lo=cpb_lo, cpb_hi=cpb_hi, ch=ch, wlo=wlo, whi=whi,
                n=n, inv_perm=inv_perm, xpad_perm_T=np.ascontiguousarray(
                    xpad_perm.T).astype(bf16))
    return meta, in_maps


def _build(meta):
    """Build the (single, SPMD-shared) Bass program."""
    from contextlib import ExitStack
    import concourse.bacc as bacc
    import concourse.mybir as mybir
    import concourse.tile as tile
    from concourse.tile import add_dep_helper

    f32 = mybir.dt.float32
    b16 = mybir.dt.bfloat16
    i16 = mybir.dt.int16
    Alu = mybir.AluOpType
    Act = mybir.ActivationFunctionType
    AxX = mybir.AxisListType.X

    nbc, sbb, nsb = meta["nbc"], meta["sbb"], meta["nsb"]
    npb, npad, nnc = meta["npb"], meta["npad"], meta["nnc"]
    cpb_lo, cpb_hi, ch = meta["cpb_lo"], meta["cpb_hi"], meta["ch"]
    wlo, whi = meta["wlo"], meta["whi"]
    ncores = meta["ncores"]

    nc = bacc.Bacc("TRN2", target_bir_lowering=False, debug=False,
                   num_swdge_queues=4)

    t_xT = nc.dram_tensor("xT_full", [D, npad], b16, kind="ExternalInput")
    t_xl = nc.dram_tensor("x_local", [nnc, D], b16, kind="ExternalInput")
    t_kvlo = nc.dram_tensor("kvlo_idx", [128, nsb * wlo // 16], i16,
                            kind="ExternalInput")
    t_kvhi = nc.dram_tensor("kvhi_idx", [128, nsb * whi // 16], i16,
                            kind="ExternalInput")
    t_ptT = nc.dram_tensor("ptT", [128, nsb * ch * BLK], b16,
                           kind="ExternalInput")
    t_wskvc = nc.dram_tensor("Wskvc", [D, 2 * D], b16, kind="ExternalInput")
    t_wt = nc.dram_tensor("Wt", [D, D], b16, kind="ExternalInput")
    t_wout = nc.dram_tensor("Wout", [D, D], b16, kind="ExternalInput")
    t_ident = nc.dram_tensor("ident", [BLK, BLK], b16, kind="ExternalInput")
    t_d0 = nc.dram_tensor("delta0", [BLK, BLK], b16, kind="ExternalInput")
    t_brow = nc.dram_tensor("bias_row", [BLK, BLK], b16, kind="ExternalInput")

    t_kv = nc.dram_tensor("kv_table", [npad, 2 * D], b16, kind="Internal")
    t_out = nc.dram_tensor("out", [nnc, D], f32, kind="ExternalOutput")

    store_insts = []

    with ExitStack() as ctx:
        tc = ctx.enter_context(tile.TileContext(nc))
        cpool = ctx.enter_context(tc.tile_pool(name="const", bufs=1))

        def load_const(t, shape, dtype):
            s = cpool.tile(shape, dtype, tag=t.name)
            nc.sync.dma_start(s[:], t[:])
            return s

        c_wskvc = load_const(t_wskvc, [D, 2 * D], b16)
        c_wt = load_const(t_wt, [D, D], b16)
        c_wout = load_const(t_wout, [D, D], b16)
        c_ident = load_const(t_ident, [BLK, BLK], b16)
        c_d0 = load_const(t_d0, [BLK, BLK], b16)
        c_brow = load_const(t_brow, [BLK, BLK], b16)
        c_kvlo = load_const(t_kvlo, list(t_kvlo.shape), i16)
        c_kvhi = load_const(t_kvhi, list(t_kvhi.shape), i16)
        # persistent Q table for the core's own nodes: [j-in-block, b, d]
        q_sb = cpool.tile([128, nbc, D], b16, tag="q_sb")

        # ---------------- Phase A: projections ----------------
        # hi-table groups (rows >= HI_BASE) first: the per-superblock hi
        # gather is issued first in phase B, so its sentinel should fire
        # as early as possible.
        t_xTl = nc.dram_tensor("xT_local", [D, nnc], b16, kind="ExternalInput")
        GA = 16
        lo_groups = LO_LIMIT // BLK // GA    # lo table = groups [0, 32)
        hi_first = HI_BASE // BLK // GA      # hi table = groups [18, 50)
        group_order = list(range(hi_first, npb // GA)) + list(range(hi_first))
        with tc.tile_pool(name="pa", bufs=3) as pa, \
             tc.tile_pool(name="pa_ps", bufs=4, space="PSUM") as pa_ps:
            # q first: q_sb is needed by the first superblock's score matmuls
            for s in range(nbc // 2):
                xq = pa.tile([128, 2 * BLK], b16, tag="xq")
                nc.sync.dma_start(xq[:], t_xTl[:, s * 2 * BLK:(s + 1) * 2 * BLK])
                for b in range(2):
                    psq = pa_ps.tile([128, BLK], f32, tag="qps")
                    nc.tensor.matmul(psq[:], xq[:, b * BLK:(b + 1) * BLK],
                                     c_wt[:], start=True, stop=True)
                    nc.scalar.copy(q_sb[:, s * 2 + b, :], psq[:])
            for g in group_order:
                xa = pa.tile([128, GA * BLK], b16, tag="xa")
                nc.sync.dma_start(xa[:], t_xT[:, g * GA * BLK:(g + 1) * GA * BLK])
                kvsb = pa.tile([128, GA * 256], b16, tag="kvsb")
                for k in range(GA // 2):
                    ps = pa_ps.tile([128, 512], f32, tag="kvps")
                    for b in range(2):
                        nc.tensor.matmul(
                            ps[:, b * 256:(b + 1) * 256],
                            xa[:, (2 * k + b) * BLK:(2 * k + b + 1) * BLK],
                            c_wskvc[:], start=True, stop=True)
                    if k % 2 == 0:
                        nc.scalar.copy(kvsb[:, k * 512:(k + 1) * 512], ps[:])
                    else:
                        nc.vector.tensor_copy(kvsb[:, k * 512:(k + 1) * 512],
                                              ps[:])
                base = g * GA * BLK
                # xT columns are host-permuted so partition p holds nodes
                # base + p*8 + b -> contiguous 4KB store per partition
                # ACT-ring HWDGE: next group's xa load on the sync ring
                # is not FIFO-blocked behind this store
                st = nc.scalar.dma_start(
                    t_kv[base: base + GA * BLK, :]
                        .rearrange("(p b) e -> p b e", p=128),
                    kvsb[:].rearrange("p (b e) -> p b e", b=GA))
                store_insts.append((g, st.ins))

        # join sentinel: all phase-B gathers depend on all phase-A kv stores
        sent_pool = ctx.enter_context(tc.tile_pool(name="sent", bufs=1))
        sent_lo = sent_pool.tile([1, 1], f32, tag="sent_lo")
        sent_hi = sent_pool.tile([1, 1], f32, tag="sent_hi")
        sj_lo = nc.vector.memset(sent_lo[:], 0.0)
        sj_hi = nc.vector.memset(sent_hi[:], 0.0)
        for gid, st in store_insts:
            if gid < lo_groups:
                add_dep_helper(sj_lo.ins, st, sync=True, reason="phaseA lo join")
            if gid >= hi_first:
                add_dep_helper(sj_hi.ins, st, sync=True, reason="phaseA hi join")

        # ---------------- Phase B: gather / attention ----------------
        kvp = ctx.enter_context(tc.tile_pool(name="kvg", bufs=4))
        ptp = ctx.enter_context(tc.tile_pool(name="ptT", bufs=2))
        wp = ctx.enter_context(tc.tile_pool(name="work", bufs=6))
        pp = ctx.enter_context(tc.tile_pool(name="pstrip", bufs=8))
        mp = ctx.enter_context(tc.tile_pool(name="msge", bufs=4))
        sp = ctx.enter_context(tc.tile_pool(name="strips", bufs=4))
        fp = ctx.enter_context(tc.tile_pool(name="fin", bufs=4))
        ps_qsel = ctx.enter_context(tc.tile_pool(name="psq", bufs=2, space="PSUM"))
        ps_pt = ctx.enter_context(tc.tile_pool(name="pspt", bufs=2, space="PSUM"))
        ps_agg = ctx.enter_context(tc.tile_pool(name="psagg", bufs=2,
                                                space="PSUM"))
        ps_fin = ctx.enter_context(tc.tile_pool(name="psfin", bufs=1, space="PSUM"))

        nlo_c = sbb * cpb_lo  # lo chunks per superblock

        for s in range(nsb):
            kvg = kvp.tile([128, ch, 2 * D], b16, tag="kvg")
            # hi gather first (its phase-A sentinel fires first)
            g2 = nc.gpsimd.dma_gather(
                out_ap=kvg[:, nlo_c:ch, :],
                in_ap=t_kv[HI_BASE:HI_BASE + LO_LIMIT, :],
                idxs_ap=c_kvhi[:, s * (whi // 16):(s + 1) * (whi // 16)],
                num_idxs=whi, num_idxs_reg=whi, elem_size=2 * D,
                queue_num=(2 * s + 1) % 4, single_packet=False)
            add_dep_helper(g2.ins, sj_hi.ins, sync=True, reason="waitA")
            g1 = nc.gpsimd.dma_gather(
                out_ap=kvg[:, 0:nlo_c, :],
                in_ap=t_kv[0:LO_LIMIT, :],
                idxs_ap=c_kvlo[:, s * (wlo // 16):(s + 1) * (wlo // 16)],
                num_idxs=wlo, num_idxs_reg=wlo, elem_size=2 * D,
                queue_num=(2 * s) % 4, single_packet=False)
            add_dep_helper(g1.ins, sj_lo.ins, sync=True, reason="waitA")

            # host-precomputed P^T strip for this superblock (HWDGE);
            # P is derived on-chip via PE transpose (DMA is the pacer)
            ptt = ptp.tile([128, ch * BLK], b16, tag="ptt")
            nc.sync.dma_start(ptt[:], t_ptT[:, s * ch * BLK:(s + 1) * ch * BLK])

            # residual x loads for the sbb blocks (early)
            xbs = []
            for bb in range(sbb):
                row0 = (s * sbb + bb) * BLK
                xb = fp.tile([128, D], b16, tag="xb", name=f"xb{s}_{bb}")
                nc.sync.dma_start(xb[:], t_xl[row0:row0 + BLK, :])
                xbs.append(xb)

            # chunk runs per block: (chunk0, nchunks, strip offset)
            def block_runs(bb):
                return [(bb * cpb_lo, cpb_lo, 0),
                        (nlo_c + bb * cpb_hi, cpb_hi, cpb_lo)]

            # scores + exp, per block; P strips via PE transpose of ptT
            chb = cpb_lo + cpb_hi
            p_strips = {}
            exp_strips = {}
            for bb in range(sbb):
                qcol = s * sbb + bb
                sc_strip = sp.tile([128, chb, H], b16, tag="scs",
                                   name=f"scs{s}_{bb}")
                for (c0, nch, so) in block_runs(bb):
                    P = pp.tile([128, nch, BLK], b16, tag="P",
                                name=f"P{s}_{bb}_{c0}")
                    p_strips[c0] = P
                    for g0 in range(0, nch, 4):
                        gn = min(4, nch - g0)
                        pt_ps = ps_pt.tile([128, 512], b16, tag="ptps")
                        for i in range(gn):
                            nc.tensor.transpose(
                                pt_ps[:, i * BLK:(i + 1) * BLK],
                                ptt[:, (c0 + g0 + i) * BLK:
                                    (c0 + g0 + i + 1) * BLK],
                                c_ident[:])
                        nc.scalar.copy(
                            P[:, g0:g0 + gn, :].rearrange("p c e -> p (c e)"),
                            pt_ps[:, 0:gn * BLK])
                        qsel = ps_qsel.tile([128, 512], f32, tag="qsel")
                        for i in range(gn):
                            nc.tensor.matmul(
                                qsel[:, i * BLK:(i + 1) * BLK],
                                ptt[:, (c0 + g0 + i) * BLK:
                                    (c0 + g0 + i + 1) * BLK],
                                q_sb[:, qcol, :],
                                start=True, stop=True)
                        qsb = wp.tile([128, 512], b16, tag="qsb")
                        nc.scalar.copy(qsb[:, 0:gn * BLK], qsel[:, 0:gn * BLK])
                        qk = wp.tile([128, 512], b16, tag="qk")
                        nc.vector.tensor_tensor(
                            qk[:, 0:gn * BLK].rearrange("p (c e) -> p c e", c=gn),
                            qsb[:, 0:gn * BLK]
                                .rearrange("p (c e) -> p c e", c=gn),
                            kvg[:, c0 + g0:c0 + g0 + gn, 0:D],
                            Alu.mult)
                        # bf16 out: all-2B operands -> DVE 2x_1P reduce;
                        # internal accumulation is f32, only the final
                        # score rounds (same magnitude as the other bf16
                        # roundings in this pipeline)
                        with nc.allow_low_precision("score rounds to bf16"):
                            nc.vector.tensor_reduce(
                                sc_strip[:, so + g0:so + g0 + gn, :]
                                    .rearrange("p c h -> p (c h)"),
                                qk[:, 0:gn * BLK]
                                    .rearrange("p (c h d) -> p (c h) d",
                                               c=gn, h=H),
                                axis=AxX, op=Alu.add)
                msge = mp.tile([128, chb, D + H], b16, tag="msge",
                               name=f"msge{s}_{bb}")
                nc.scalar.activation(
                    msge[:, :, D:D + H],
                    sc_strip[:].rearrange("p c h -> p (c h)"), Act.Exp)
                exp_strips[bb] = msge

            # msg + aggregation per block
            for bb in range(sbb):
                agg = ps_agg.tile([128, D + H], f32, tag="agg",
                                  name=f"agg{s}_{bb}")
                runs = block_runs(bb)
                msge = exp_strips[bb]
                for (c0, nch, so) in runs:
                    # v is (d,h)-interleaved: broadcast exp along the middle
                    # (d) axis; inner (h) axis is stride-1 -> 2x DVE mode
                    nc.vector.tensor_tensor(
                        msge[:, so:so + nch, 0:D]
                            .rearrange("p c (d h) -> p c d h", d=HD),
                        kvg[:, c0:c0 + nch, D:2 * D]
                            .rearrange("p c (d h) -> p c d h", d=HD),
                        msge[:, so:so + nch, D:D + H]
                            .unsqueeze(2).broadcast_to([128, nch, HD, H]),
                        Alu.mult)
                nmm = sum(nch for (_, nch, _) in runs)
                im = 0
                for (c0, nch, so) in runs:
                    P = p_strips[c0]
                    for i in range(nch):
                        nc.tensor.matmul(agg[:], P[:, i, :],
                                         msge[:, so + i, :],
                                         start=(im == 0), stop=(im == nmm - 1))
                        im += 1

                # ---- block epilogue ----
                row0 = (s * sbb + bb) * BLK
                rd = fp.tile([128, H], f32, tag="rd")
                nc.vector.tensor_scalar(rd[:], agg[:, D:D + H], 1e-30,
                                        None, Alu.add)
                nc.vector.reciprocal(rd[:], rd[:])
                aggn = fp.tile([128, D], b16, tag="aggn")
                nc.vector.tensor_tensor(
                    aggn[:].rearrange("p (d h) -> p d h", d=HD),
                    agg[:, 0:D].rearrange("p (d h) -> p d h", d=HD),
                    rd[:].unsqueeze(1).broadcast_to([128, HD, H]),
                    Alu.mult)
                aTp = ps_fin.tile([128, D], b16, tag="aTp")
                nc.tensor.transpose(aTp[:], aggn[:], c_ident[:])
                aT = fp.tile([128, D], b16, tag="aT")
                nc.scalar.copy(aT[:], aTp[:])
                op = ps_fin.tile([128, D], f32, tag="op")
                nc.tensor.matmul(op[:], c_d0[:], c_brow[:],
                                 start=True, stop=False)
                nc.tensor.matmul(op[:], aT[:], c_wout[:],
                                 start=False, stop=True)
                rl = fp.tile([128, D], f32, tag="rl")
                nc.scalar.activation(rl[:], op[:], Act.Relu)
                fin = fp.tile([128, D], f32, tag="fin")
                nc.vector.tensor_add(fin[:], rl[:], xbs[bb][:])
                nc.sync.dma_start(t_out[row0:row0 + BLK, :], fin[:])

    nc.compile()
    return nc


def _add_xtl(meta, in_maps):
    """Per-core transposed local x slice (bf16, slot order) for q."""
    nnc = meta["nnc"]
    xpt = meta["xpad_perm_T"]
    for c, m in enumerate(in_maps):
        first = c * nnc
        m["xT_local"] = np.ascontiguousarray(xpt[:, first:first + nnc])
    return in_maps


def _run_hw(nc, in_maps, trace=False):
    from concourse import bass_utils
    res = bass_utils.run_bass_kernel_spmd(
        nc, in_maps, core_ids=list(range(len(in_maps))), trace=trace)
    outs = [r["out"] for r in res.results]
    return outs, res


def kernel_custom(inputs, ncores=8, nbc=50, sbb=2, mode="hw", trace=False):
    meta, in_maps = _prep(
        inputs["x"], inputs["edge_index"], inputs["Wt"], inputs["Ws"],
        inputs["Wc"], inputs["Wout"], inputs["bout"], ncores, nbc, sbb)
    in_maps = _add_xtl(meta, in_maps)
    nc = _build(meta)
    outs, res = _run_hw(nc, in_maps, trace=trace)
    slots = np.concatenate(outs, axis=0)
    inv = meta["inv_perm"]
    full = np.empty((meta["n"], D), np.float32)
    valid = inv < meta["n"]
    full[inv[valid]] = slots[valid]
    return full, res


def kernel(**inputs):
    out, _ = kernel_custom(inputs, ncores=8, nbc=50, sbb=2, mode="hw")
    return out
